# revision 64
# baseline (speedup 1.0000x reference)
"""Trainium2 Bass kernel for LocalNodeAttentionHeadSum (v7).

Computation (per batch b, pixel p=(h,w)):
    q[d,p]   = sum_c x[c,TMID,p] Wq[c,d] + bq[d]
    k[t,d]   = sum_c nodes[t,c] Wk[c,d] + bk[d]
    s[t,p]   = sum_d q[d,p] k[t,d];  alpha = softmax_t(s)
    y[d,p]   = sum_t alpha[t,p] * (sum_c x[c,t,p] Wv[c,d] + bv[d])
    out[c,p] = sum_d y[d,p] Wo[d,c] + bo[c]

Weight-only algebra folded on the host:
    kT  = nodes @ Wk + bk;  Wqk = Wq @ kT.T;  sb0 = kT @ bq
    Wf  = Wv @ Wo (bf16)    [valid: sum_t alpha = 1 commutes the temporal
                             sum past the pointwise value projection]
    bo_e = bv @ Wo + bo     (added on host post-gather)

Input staging (host): the middle frame is fed fp32 (scores amplify
rounding through the exp); the six non-mid frames are fed as bf16
DELTAS against the mid frame (y = x_mid + sum_t alpha_t (x_t - x_mid)),
which halves the dominant HBM stream and drops the mid-frame multiply
entirely. Output is stored bf16 c-major [C, BL, HWF] (784B descriptor
runs keep the DMA model at full bandwidth); host upcasts + bias +
transposes back.

Device per pair of batches: fp32 score matmuls over both batches at
once ([7, 392] psum, mid DMA split 4 ways so they start early),
pair-wide softmax (max-sub on Pool, exp on ACT with the table
pre-warmed at t=0), alpha broadcast to 128 partitions via indicator
matmuls, the mid pair ACT-cast to bf16 once, then a 6-frame delta
mul + shallow add tree (mul/s1 per batch on DVE; s2 and the
independent v = s1[2]+midb on Pool for early chunks; the final join on
the engine that keeps the tail short), and the fused [C->C] bf16
projection into 8 per-pair psum banks. The Pool-chained chunk (q2)
arrives LAST so each pair's psum groups close on cc4/cc5 right after
its fast all-DVE chunks; per-pair merged stores ride the ACT HWDGE
queue so they slot between load transfers.

Sharding: data-parallel over batch B=32 across 8 cores (4 per core).
DMA floor per core ~46.5us (3.2MB mid f32 + 9.6MB delta bf16 + 2MB Wf
in, 1.6MB out bf16 at 360GB/s).
"""

import sys

for _p in ("/opt/trn_rl_repo",):
    if _p not in sys.path:
        sys.path.insert(0, _p)

from contextlib import ExitStack

import numpy as np

import concourse.bass as bass
import concourse.tile as tile
from concourse import bacc, mybir, bass_isa
from concourse.bass_utils import run_bass_kernel_spmd

F32 = mybir.dt.float32
F32R = mybir.dt.float32r
BF16 = mybir.dt.bfloat16

# Problem shapes (hardcoded per contract)
B, C, T, H, W = 32, 1024, 7, 14, 14
D = 512
NCORES = 8
BL = B // NCORES          # 4 batches per core
HWF = H * W               # 196
CC = C // 128             # 8 chunks over channels
TMID = T // 2             # 3 (middle frame)
F2 = 2 * HWF              # 392: the two batches of a pair along free axis
RESTN = 6                 # non-mid frames
REST = RESTN * HWF        # 1176
HALF = 3 * HWF            # 588
TF = T * HWF              # 1372: all frames of one chunk

Exp = mybir.ActivationFunctionType.Exp

# f32r runs the score matmuls at 1 cycle/row (vs 4 for fp32); flip to
# False if hardware f32r output is off.
F32R_SCORES = False

# rest stages: (first chunk, last chunk) — uneven so the tail is small
QCH = [(0, 2), (2, 4), (4, 6), (6, 7), (7, 8)]
NQ = len(QCH)

# which tree stages run on Pool per chunk (rebalance DVE); the last
# chunks stay all-DVE so the tail drains fast
POOL_STAGES = {
    0: {"s2", "v", "xw"},
    1: {"s2", "v", "xw"},
    2: {"s2", "v"},
    3: set(),
    4: set(),
}


def build_program():
    nc = bacc.Bacc("TRN2", target_bir_lowering=False, debug=False)

    xm_d = nc.dram_tensor("x_mid", [BL, C, HWF], F32, kind="ExternalInput").ap()
    xr_d = nc.dram_tensor("x_rest", [BL, C, REST], BF16, kind="ExternalInput").ap()
    wf_d = nc.dram_tensor("Wf", [C, C], BF16, kind="ExternalInput").ap()
    wqk_d = nc.dram_tensor("Wqk", [C, T], F32, kind="ExternalInput").ap()
    sb0_d = nc.dram_tensor("sb0", [1, T], F32, kind="ExternalInput").ap()
    out_d = nc.dram_tensor("out", [C, BL, HWF], BF16, kind="ExternalOutput").ap()

    xm_r = xm_d.rearrange("b (cc p) f -> b p cc f", p=128)
    xr_r = xr_d.rearrange("b (cc p) s -> b p cc s", p=128)
    wf_r = wf_d.rearrange("(cc p) c2 -> p cc c2", p=128)
    wqk_r = wqk_d.rearrange("(cc p) t -> p cc t", p=128)
    out_r = out_d.rearrange("(ccp p) b f -> p ccp (b f)", p=128)

    with tile.TileContext(nc) as tc, ExitStack() as ctx:
        cpool = ctx.enter_context(tc.tile_pool(name="const", bufs=1))
        midpool = ctx.enter_context(tc.tile_pool(name="mid", bufs=2))
        midbpool = ctx.enter_context(tc.tile_pool(name="midb", bufs=2))
        restpool = ctx.enter_context(tc.tile_pool(name="rest", bufs=3))
        rest1pool = ctx.enter_context(tc.tile_pool(name="rest1", bufs=2))
        abpool = ctx.enter_context(tc.tile_pool(name="ab", bufs=2))
        xwpool = ctx.enter_context(tc.tile_pool(name="xw", bufs=2))
        tmpool = ctx.enter_context(tc.tile_pool(name="tm", bufs=3))
        s1pool = ctx.enter_context(tc.tile_pool(name="s1", bufs=3))
        s2pool = ctx.enter_context(tc.tile_pool(name="s2", bufs=2))
        smpool = ctx.enter_context(tc.tile_pool(name="sm", bufs=2))
        obpool = ctx.enter_context(tc.tile_pool(name="ob", bufs=1))
        psp = ctx.enter_context(tc.tile_pool(name="ps", bufs=1, space="PSUM"))

        # ---- constants (SWDGE queue; SP stays clear for the x stream) ----
        warmrhs = cpool.tile([T, F2], BF16)
        nc.gpsimd.memset(warmrhs[:], 0.0)
        wones7c = cpool.tile([T, 1], BF16)
        nc.gpsimd.memset(wones7c[:], 1.0)
        # const loads on the ACT HWDGE queue: Pool's engine/FIFO stays clear
        wqk_sb = cpool.tile([128, CC * T], F32)
        nc.scalar.dma_start(
            wqk_sb[:].rearrange("p (cc t) -> p cc t", t=T), wqk_r
        )
        sb0_sb = cpool.tile([1, T], F32)
        nc.scalar.dma_start(sb0_sb[:], sb0_d)
        import ml_dtypes

        e_np = np.zeros((T, T * 128), dtype=ml_dtypes.bfloat16)
        for t in range(T):
            e_np[t, t * 128 : (t + 1) * 128] = 1.0
        e_dram = nc.inline_tensor(e_np, name="e_ind")
        e_all = cpool.tile([T, T * 128], BF16)
        nc.scalar.dma_start(e_all[:], e_dram.ap())
        Es = [e_all[:, t * 128 : (t + 1) * 128] for t in range(T)]

        ones392 = cpool.tile([1, F2], F32)
        nc.gpsimd.memset(ones392[:], 1.0)
        ones7c = cpool.tile([T, 1], BF16)
        nc.gpsimd.memset(ones7c[:], 1.0)
        ones128 = cpool.tile([1, 128], F32)
        nc.gpsimd.memset(ones128[:], 1.0)
        # warm the ACT exp table at t=0 so the 1.3us table load is off the
        # softmax critical chain
        actwarm = cpool.tile([1, 8], F32)
        nc.gpsimd.memset(actwarm[:], 0.0)
        actwarm2 = cpool.tile([1, 8], BF16)
        nc.scalar.activation(actwarm2[:], actwarm[:], Exp, bias=0.0, scale=1.0)

        # Wf in four independent tiles (2 chunks each) so each piece's DMA
        # interleaves with the x stream and unblocks its proj chunks alone
        wf_sb = [cpool.tile([128, 2 * C], BF16, name=f"wf{i}") for i in range(4)]

        state = [dict() for _ in range(2)]  # per pair

        # Psum tiles rotate through 8 one-bank slots: pj0-3 + pk0-3.
        _rot = [0]
        _rotj = [0]

        def psum_sm(shape, fam="pk"):
            r = _rot if fam == "pk" else _rotj
            t = psp.tile(shape, F32, tag=f"{fam}{r[0]}", bufs=1, name=f"smps{fam}{r[0]}")
            r[0] = (r[0] + 1) % 4
            return t

        def sc(ap):
            return ap.bitcast(F32R) if F32R_SCORES else ap

        # ---- stage emitters -------------------------------------------
        def emit_mid(pr):
            # per-pair tile, layout (cc, l, f) so score rhs is [128, F2];
            # split into (l, cc-half) DMAs so the first score matmuls can
            # start on a quarter-loaded tile
            mid = midpool.tile([128, CC * F2], F32, tag="mid")
            mv = mid[:].rearrange("p (cc l f) -> p cc (l f)", cc=CC, l=2)
            for h in range(2):
                cs = slice(h * 4, (h + 1) * 4)
                for l in range(2):
                    b = 2 * pr + l
                    nc.sync.dma_start(
                        mv[:, cs, l * HWF : (l + 1) * HWF], xm_r[b][:, cs]
                    )
            state[pr]["mid"] = mid

        def emit_wf(i):
            nc.sync.dma_start(
                wf_sb[i][:].rearrange("p (cc c2) -> p cc c2", c2=C),
                wf_r[:, 2 * i : 2 * i + 2],
            )

        def wfs(cc):
            return wf_sb[cc // 2][:, (cc % 2) * C : (cc % 2 + 1) * C]

        def emit_rest(pr, q):
            """6-frame bf16 delta tile (x_t - x_mid), layout (l, cc, s)."""
            c0, c1 = QCH[q]
            n = c1 - c0
            pool = restpool if n == 2 else rest1pool
            rq = pool.tile([128, 2 * n * REST], BF16, tag=f"rest{n}", name="rq")
            state[pr][f"rq{q}"] = rq
            rv = rq[:].rearrange("p (l cc s) -> p l cc s", l=2, cc=n)
            for l in range(2):
                b = 2 * pr + l
                nc.sync.dma_start(rv[:, l], xr_r[b][:, c0:c1, :])

        def emit_midcast(pr, half=None):
            """ACT-cast the fp32 mid pair to bf16 (the identity term of
            y = x_mid + sum_t alpha_t (x_t - x_mid)); split in halves so
            the ACT engine is never hogged for 2.6us straight."""
            st = state[pr]
            if half in (None, 0):
                st["midb"] = midbpool.tile(
                    [128, CC * F2], BF16, tag="midb", name="midb"
                )
            midb = st["midb"]
            hs = [0, 1] if half is None else [half]
            for h in hs:
                sl = slice(h * 4 * F2, (h + 1) * 4 * F2)
                nc.scalar.copy(midb[:, sl], st["mid"][:, sl])

        def emit_scores(pr):
            """Pair-wide f32r score matmuls into a [7, 392] psum tile."""
            st = state[pr]
            mid = st["mid"]
            st["ab"] = abpool.tile([128, RESTN * F2], BF16, tag="ab", name="ab")
            st["xw"] = xwpool.tile([128, CC * F2], BF16, tag="xw", name="xw")
            sp = psum_sm([T, F2], "pj" if pr == 1 else "pk")
            for cc in range(CC):
                nc.tensor.matmul(
                    sp[:],
                    sc(wqk_sb[:, cc * T : (cc + 1) * T]),
                    sc(mid[:, cc * F2 : (cc + 1) * F2]),
                    start=(cc == 0),
                    stop=False,
                )
            nc.tensor.matmul(sp[:], sc(sb0_sb[:]), sc(ones392[:]), start=False, stop=True)
            st["sp"] = sp
            st["zp"] = psum_sm([1, F2], "pj" if pr == 1 else "pk")

        def emit_sm(pr):
            """Max-subtracted exponentials only — the 1/Z normalization is
            deferred to a per-chunk rescale in the tree tail, so the value
            combine can start without waiting for sum/reciprocal. Pair 1's
            subtract runs on Pool so it never blocks the DVE tree FIFO."""
            st = state[pr]
            sp = st["sp"]
            s_sb = smpool.tile([T, F2], F32, tag="ssb", bufs=1)
            nc.scalar.copy(s_sb[:], sp[:])
            mx = smpool.tile([T, F2], F32, tag="mx", bufs=1)
            nc.gpsimd.partition_all_reduce(
                mx[:], s_sb[:], channels=T, reduce_op=bass_isa.ReduceOp.max
            )
            sm = smpool.tile([T, F2], F32, tag="smx", bufs=1)
            # on Pool so it never blocks the DVE tree FIFO
            nc.gpsimd.tensor_sub(sm[:], s_sb[:], mx[:])
            e_sb = smpool.tile([T, F2], BF16, tag="e")
            nc.scalar.activation(e_sb[:], sm[:], Exp, bias=0.0, scale=1.0)
            st["e"] = e_sb

        def emit_rz(pr):
            """Z = sum_t e_t, 1/Z, then alpha = e/Z (the mid identity term
            makes the tree need only the six normalized non-mid rows)."""
            st = state[pr]
            zp, e_sb = st["zp"], st["e"]
            nc.tensor.matmul(zp[:], ones7c[:], e_sb[:], start=True, stop=True)
            rz = smpool.tile([1, F2], F32, tag="rz", bufs=1)
            nc.vector.reciprocal_approx_fast(rz[:], zp[:])
            rb = smpool.tile([T, F2], F32, tag="rb", bufs=1)
            nc.gpsimd.partition_broadcast(rb[:], rz[:])
            aT = smpool.tile([T, F2], BF16, tag="aT")
            if pr == 0:
                nc.vector.tensor_mul(aT[:], e_sb[:], rb[:])
            else:
                # keep pair 1's chain out of the DVE tree FIFO
                nc.gpsimd.tensor_mul(aT[:], e_sb[:], rb[:])
            st["aT"] = aT

        TMAP = [0, 1, 2, 4, 5, 6]  # ab position -> t (mid handled via identity)

        def emit_bc(pr, l):
            """Broadcast the six non-mid alpha rows to 128 partitions:
            three adjacent position-pairs, one psum bank each; copies split
            ACT/DVE to halve the chain hop."""
            st = state[pr]
            ab, e_sb = st["ab"], st["aT"]
            lsl = slice(l * HWF, (l + 1) * HWF)
            fam = "pk"
            abv = ab[:].rearrange("p (t f) -> p t f", t=RESTN)
            for ci, pi in enumerate((0, 2, 4)):
                pp = psum_sm([128, F2], fam)
                for k in range(2):
                    nc.tensor.matmul(
                        pp[:, k * HWF : (k + 1) * HWF],
                        Es[TMAP[pi + k]],
                        e_sb[:, lsl],
                        start=True,
                        stop=True,
                    )
                dst = abv[:, pi : pi + 2, lsl]
                src = pp[:].rearrange("p (t f) -> p t f", t=2)
                if ci == 1 and pr == 0:
                    nc.vector.tensor_copy(dst, src)
                else:
                    nc.scalar.copy(dst, src)

        def emit_tree(pr, q, only_l=None):
            """6-frame delta mul (raw-exp weights) + add tree over both
            batches; tail rescales by 1/Z and adds the mid identity term."""
            st = state[pr]
            ab, xw, rq = st["ab"], st["xw"], st[f"rq{q}"]
            midb = st["midb"]
            c0, c1 = QCH[q]
            n = c1 - c0
            ls = slice(0, 2) if only_l is None else slice(only_l, only_l + 1)
            nl = 2 if only_l is None else 1
            pst = POOL_STAGES[q]

            def stage_op(s, out, a, b, mul=False):
                eng = nc.gpsimd if s in pst else nc.vector
                if mul:
                    eng.tensor_mul(out, a, b)
                else:
                    eng.tensor_add(out, a, b)
            rql = rq[:].rearrange(
                "p (l cc t f) -> p l cc t f", l=2, cc=n, t=RESTN
            )
            tm = tmpool.tile([128, 2 * n * REST], BF16, tag="tm", name="tm")
            tvl = tm[:].rearrange(
                "p (l cc t f) -> p l cc t f", l=2, cc=n, t=RESTN
            )
            s1 = s1pool.tile([128, 2 * n * HALF], BF16, tag="s1", name="s1")
            s1f = s1[:].rearrange(
                "p (l cc t f) -> p l cc t f", l=2, cc=n, t=TMID
            )
            # per-batch ops: the HW ISA caps free dims at 3
            abl = ab[:].rearrange("p (t l f) -> p l t f", t=RESTN, l=2)
            for lx in ([0, 1] if only_l is None else [only_l]):
                nc.vector.tensor_mul(
                    tvl[:, lx],
                    rql[:, lx],
                    abl[:, lx].unsqueeze(1).broadcast_to((128, n, RESTN, HWF)),
                )
                nc.vector.tensor_add(
                    s1f[:, lx], tvl[:, lx, :, 0:3], tvl[:, lx, :, 3:6]
                )
            # tail stages emitted per batch with <=3-dim views (the HW
            # verifier caps TensorScalarPtr access patterns at 3 dims)
            s2 = s2pool.tile([128, 2 * n * HWF], BF16, tag="s2", name="s2")
            s2l = s2[:].rearrange("p (l cc f) -> p l cc f", l=2, cc=n)
            vt = s2pool.tile([128, 2 * n * HWF], BF16, tag="v", name="v")
            vl = vt[:].rearrange("p (l cc f) -> p l cc f", l=2, cc=n)
            s1l = s1f
            xwl = xw[:].rearrange("p (cc l f) -> p l cc f", cc=CC, l=2)
            mbl = midb[:].rearrange("p (cc l f) -> p l cc f", cc=CC, l=2)
            lrange = [0, 1] if only_l is None else [only_l]
            for lx in lrange:
                stage_op("s2", s2l[:, lx], s1l[:, lx, :, 0], s1l[:, lx, :, 1])
                # v is independent of s2 (reads s1 + midb): shallower chain
                stage_op("v", vl[:, lx], s1l[:, lx, :, 2], mbl[:, lx, c0:c1])
                stage_op("xw", xwl[:, lx, c0:c1], s2l[:, lx], vl[:, lx])

        def emit_proj_alloc():
            """8 psum tiles per pair (one bank each): tile ccp holds output
            block ccp for BOTH batches, layout (l, f)."""
            return [
                psp.tile(
                    [128, F2],
                    F32,
                    tag=("pj" if ccp < 4 else "pk") + str(ccp % 4),
                    bufs=1,
                    name=f"pt{ccp}",
                )
                for ccp in range(CC)
            ]

        def emit_proj_chunk(ptiles, pr, cc, start, stop):
            xw = state[pr]["xw"]
            rhs = xw[:, cc * F2 : (cc + 1) * F2]
            w = wfs(cc)
            for ccp in range(CC):
                nc.tensor.matmul(
                    ptiles[ccp][:],
                    w[:, ccp * 128 : (ccp + 1) * 128],
                    rhs,
                    start=start,
                    stop=stop,
                )

        def emit_ob_alloc(pr):
            obs = [
                obpool.tile([128, 2 * F2], BF16, tag=f"ob{jj}", name=f"ob{jj}")
                for jj in range(4)
            ]
            state[pr]["obs"] = obs
            return obs

        def emit_ob_st(ptiles, pr):
            # copies split across ACT/DVE; each jj's store right after its
            # two copies so the tail overlaps
            obs = state[pr]["obs"]
            for jj in range(4):
                for k in range(2):
                    dst = obs[jj][:, k * F2 : (k + 1) * F2]
                    src = ptiles[2 * jj + k][:]
                    if k == 1 and pr == 1:
                        nc.vector.tensor_copy(dst, src)
                    else:
                        nc.scalar.copy(dst, src)
                nc.scalar.dma_start(
                    out_r[:, 2 * jj : 2 * jj + 2, pr * F2 : (pr + 1) * F2],
                    obs[jj][:].rearrange("p (k s) -> p k s", s=F2),
                )

        # ---- emission schedule ----------------------------------------
        warmp = psum_sm([1, F2])
        for _ in range(16):
            nc.tensor.matmul(warmp[:], wones7c[:], warmrhs[:], start=True, stop=True)
        emit_mid(0)
        emit_rest(0, 0)
        emit_mid(1)
        emit_scores(0)
        emit_sm(0)
        emit_rz(0)
        emit_bc(0, 0)
        emit_bc(0, 1)
        emit_wf(0)
        emit_rest(0, 1)
        emit_scores(1)
        emit_sm(1)
        emit_midcast(0)
        emit_tree(0, 0)
        emit_rz(1)
        emit_bc(1, 0)
        emit_bc(1, 1)
        pt_p0 = emit_proj_alloc()
        emit_proj_chunk(pt_p0, 0, 0, start=True, stop=False)
        emit_proj_chunk(pt_p0, 0, 1, start=False, stop=False)
        emit_wf(1)
        emit_midcast(1)
        emit_tree(0, 1)
        emit_proj_chunk(pt_p0, 0, 2, start=False, stop=False)
        emit_proj_chunk(pt_p0, 0, 3, start=False, stop=False)
        emit_wf(3)
        emit_rest(0, 3)
        emit_tree(0, 3)
        emit_proj_chunk(pt_p0, 0, 6, start=False, stop=False)
        emit_wf(2)
        emit_rest(0, 4)
        emit_tree(0, 4)
        emit_proj_chunk(pt_p0, 0, 7, start=False, stop=False)
        emit_rest(0, 2)
        emit_tree(0, 2)
        emit_proj_chunk(pt_p0, 0, 4, start=False, stop=False)
        emit_proj_chunk(pt_p0, 0, 5, start=False, stop=True)
        emit_ob_alloc(0)
        emit_rest(1, 0)
        emit_ob_st(pt_p0, 0)
        pt_p1 = emit_proj_alloc()
        emit_tree(1, 0)
        emit_proj_chunk(pt_p1, 1, 0, start=True, stop=False)
        emit_proj_chunk(pt_p1, 1, 1, start=False, stop=False)
        emit_rest(1, 1)
        emit_tree(1, 1)
        emit_proj_chunk(pt_p1, 1, 2, start=False, stop=False)
        emit_proj_chunk(pt_p1, 1, 3, start=False, stop=False)
        emit_rest(1, 3)
        emit_rest(1, 4)
        emit_tree(1, 3)
        emit_proj_chunk(pt_p1, 1, 6, start=False, stop=False)
        emit_rest(1, 2)
        emit_tree(1, 4, only_l=0)
        emit_tree(1, 4, only_l=1)
        emit_proj_chunk(pt_p1, 1, 7, start=False, stop=False)
        emit_tree(1, 2)
        emit_proj_chunk(pt_p1, 1, 4, start=False, stop=False)
        emit_proj_chunk(pt_p1, 1, 5, start=False, stop=True)
        emit_ob_alloc(1)
        emit_ob_st(pt_p1, 1)

    nc.compile()
    return nc


_PROG = None


def _get_prog():
    global _PROG
    if _PROG is None:
        _PROG = build_program()
    return _PROG


REST_IDX = [0, 1, 2, 4, 5, 6]


def _shard_inputs(inputs):
    import ml_dtypes

    f = lambda k: np.asarray(inputs[k], dtype=np.float64)
    x = np.asarray(inputs["x_window"], dtype=np.float32).reshape(B, C, T, HWF)
    nodes, Wq, bq, Wk, bk = f("nodes"), f("Wq"), f("bq"), f("Wk"), f("bk")
    Wv, bv, Wo, bo = f("Wv"), f("bv"), f("Wo"), f("bo")
    kT = nodes @ Wk + bk                                   # [T, D]
    shared = {
        "Wqk": np.ascontiguousarray((Wq @ kT.T).astype(np.float32)),
        "sb0": np.ascontiguousarray((kT @ bq).astype(np.float32).reshape(1, T)),
        "Wf": np.ascontiguousarray((Wv @ Wo).astype(ml_dtypes.bfloat16)),
    }
    bo_e = (bv @ Wo + bo).astype(np.float32)               # [C] host-added
    xm = np.ascontiguousarray(x[:, :, TMID])               # [B, C, HWF] fp32
    # deltas vs the mid frame: y = x_mid + sum_t alpha_t (x_t - x_mid)
    xr = np.ascontiguousarray(
        (x[:, :, REST_IDX] - x[:, :, TMID : TMID + 1])
        .reshape(B, C, REST)
        .astype(ml_dtypes.bfloat16)
    )
    in_maps = []
    for i in range(NCORES):
        m = dict(shared)
        m["x_mid"] = np.ascontiguousarray(xm[i * BL : (i + 1) * BL])
        m["x_rest"] = np.ascontiguousarray(xr[i * BL : (i + 1) * BL])
        in_maps.append(m)
    return in_maps, bo_e


def _postprocess(outs, bo_e):
    # outs: per-core [C, BL, HWF] bf16 -> [nb, C, 1, H, W] fp32 + bias
    full = np.concatenate([np.asarray(o) for o in outs], axis=1).astype(np.float32)
    full += bo_e[:, None, None]
    nb = full.shape[1]
    return np.ascontiguousarray(
        full.transpose(1, 0, 2).reshape(nb, C, 1, H, W)
    )


def kernel(**inputs):
    nc = _get_prog()
    in_maps, bo_e = _shard_inputs(inputs)
    res = run_bass_kernel_spmd(nc, in_maps, core_ids=list(range(NCORES)))
    return _postprocess([res.results[i]["out"] for i in range(NCORES)], bo_e)
